# revision 1
# baseline (speedup 1.0000x reference)
"""Masked multi-head self-attention kernel for 8 Trainium2 NeuronCores.

Full module: qkv projection -> causal softmax attention (16 heads) -> out
projection, for x[4, 2048, 1024].

Sharding: core c handles batch b = c//2 and heads h0 = (c%2)*8 .. h0+8.
QKV projection + attention are fully local to a core.  The out projection
contracts over all 16 heads' channels, so the two cores of a batch exchange
their attention outputs with pairwise AllGathers (chunked over heads and
query blocks for overlap) and each computes half of the output columns.
Each core returns out[b][:, half].T (transposed: [512, 2048]); the host
reassembles.  Inputs are re-laid-out per core on the host: x transposed,
qkv weight columns / out-proj rows sliced and permuted to the gather order.
"""

import math
import os
import sys

for _p in ("/opt/trn_rl_repo", "/root/.axon_site/_ro/trn_rl_repo"):
    if os.path.isdir(_p) and _p not in sys.path:
        sys.path.insert(0, _p)
        break

import ml_dtypes
import numpy as np

import concourse.bass as bass
import concourse.mybir as mybir
import concourse.tile as tile
from concourse import bacc
from concourse.bass_utils import run_bass_kernel_spmd

B, T, C, H = 4, 2048, 1024, 16
D = 64                 # head dim
NCORES = 8
HPC = H // 2           # heads per core = 8
CPC = HPC * D          # channels per core = 512
P = 128                # partitions
QB = 512               # query block
NQB = T // QB          # 4
KC = C // P            # contraction chunks for C = 8
NTT = T // P           # 16 t-tiles
SCALE = 1.0 / math.sqrt(D)

F32 = mybir.dt.float32
F32R = mybir.dt.float32r
BF16 = mybir.dt.bfloat16
EXP = mybir.ActivationFunctionType.Exp

_CACHE = {}


def build():
    nc = bacc.Bacc("TRN2", num_devices=NCORES, debug=False)

    xT = nc.dram_tensor("xT", [C, T], BF16, kind="ExternalInput")
    wqkv = nc.dram_tensor("wqkv", [C, 3 * CPC], BF16, kind="ExternalInput")
    bqkv = nc.dram_tensor("bqkv", [1, 3 * CPC], F32, kind="ExternalInput")
    wout = nc.dram_tensor("wout", [C, CPC], BF16, kind="ExternalInput")
    bout = nc.dram_tensor("bout", [1, CPC], F32, kind="ExternalInput")
    outT = nc.dram_tensor("outT", [CPC, T], F32, kind="ExternalOutput")

    groups = [[0, 1], [2, 3], [4, 5], [6, 7]]

    with tile.TileContext(nc) as tc:
        with (
            tc.tile_pool(name="const", bufs=1) as constp,
            tc.tile_pool(name="ytp", bufs=1) as ytp,
            tc.tile_pool(name="vaugp", bufs=1) as vaugp,
            tc.tile_pool(name="dram", bufs=1, space="DRAM") as dramp,
        ):
            # per-partition bias layouts: bq_sb[p, n] = bqkv[n*128 + p]
            bq_sb = constp.tile([P, 12], F32, tag="bq")
            nc.sync.dma_start(
                bq_sb[:].rearrange("p (o n) -> p o n", o=1),
                bqkv.ap().rearrange("o (n p) -> p o n", p=P),
            )
            bo_sb = constp.tile([P, 4], F32, tag="bo")
            nc.sync.dma_start(
                bo_sb[:].rearrange("p (o n) -> p o n", o=1),
                bout.ap().rearrange("o (n p) -> p o n", p=P),
            )
            ones_f32 = constp.tile([P, P], F32, tag="ones")
            nc.vector.memset(ones_f32[:], 1.0)

            # Q^T,K^T: 8 chunks of [128 ch, 2048 t] (Q: 0-3, K: 4-7)
            yts = [
                ytp.tile([P, T], BF16, name=f"yt{n}", tag=f"yt{n}")
                for n in range(8)
            ]
            # V natural (+ones col) per head: ktile k at cols k*65
            vaugs = [
                vaugp.tile([P, NTT * 65], BF16, name=f"vaug{h}", tag=f"vaug{h}")
                for h in range(HPC)
            ]
            vaug3s = [
                v[:].rearrange("p (k c) -> p k c", c=65) for v in vaugs
            ]
            for h in range(HPC):
                nc.vector.tensor_copy(
                    vaug3s[h][:, :, 64:65],
                    ones_f32[:, 0:NTT].rearrange("p (a b) -> p a b", b=1),
                )

            # ---------------- stage 1: qkv projection, V ----------------
            with (
                tc.tile_pool(name="xtp", bufs=1) as xtp,
                tc.tile_pool(name="wtile", bufs=10) as wtp,
                tc.tile_pool(name="wvp", bufs=1) as wvp,
                tc.tile_pool(name="ps_y", bufs=4, space="PSUM") as psy,
                tc.tile_pool(name="ps_v", bufs=4, space="PSUM") as psv,
            ):
                # x^T chunks resident in SBUF: [128 ch, 2048 t] each
                xts = [
                    xtp.tile([P, T], BF16, name=f"xt{cc}", tag=f"xt{cc}")
                    for cc in range(KC)
                ]
                for cc in range(KC):
                    nc.sync.dma_start(
                        xts[cc][:], xT[cc * P:(cc + 1) * P, :]
                    )

                wv_tiles = []
                for kc in range(KC):
                    wv = wvp.tile(
                        [P, CPC], BF16, name=f"wv{kc}", tag=f"wv{kc}"
                    )
                    nc.sync.dma_start(
                        wv[:], wqkv[kc * P:(kc + 1) * P, 2 * CPC:3 * CPC]
                    )
                    wv_tiles.append(wv)

                def qk_chunk(n):
                    # kc outer so each weight tile serves 4 matmuls
                    pys = [
                        psy.tile([P, QB], F32, name=f"py{n}_{i}", tag="py")
                        for i in range(4)
                    ]
                    for kc in range(KC):
                        wt = wtp.tile([P, P], BF16, tag="wt")
                        nc.sync.dma_start(
                            wt[:],
                            wqkv[kc * P:(kc + 1) * P, n * P:(n + 1) * P],
                        )
                        for tc4 in range(4):
                            nc.tensor.matmul(
                                pys[tc4][:],
                                wt[:],
                                xts[kc][:, tc4 * QB:(tc4 + 1) * QB],
                                start=(kc == 0),
                                stop=(kc == KC - 1),
                            )
                    for tc4 in range(4):
                        nc.vector.tensor_scalar_add(
                            yts[n][:, tc4 * QB:(tc4 + 1) * QB],
                            pys[tc4][:],
                            bq_sb[:, n:n + 1],
                        )

                def v_block(tts):
                    # V natural: out[t, vch] with x^T tiles stationary;
                    # V bias is folded into the output bias on the host
                    for tt in tts:
                        pv = psv.tile([P, CPC], F32, tag="pv")
                        for kc in range(KC):
                            nc.tensor.matmul(
                                pv[:],
                                xts[kc][:, tt * P:(tt + 1) * P],
                                wv_tiles[kc][:],
                                start=(kc == 0),
                                stop=(kc == KC - 1),
                            )
                        for h in range(HPC):
                            nc.vector.tensor_copy(
                                vaug3s[h][:, tt, 0:64],
                                pv[:, h * 64:h * 64 + 64],
                            )

                for blk in range(4):
                    qk_chunk(blk)
                    qk_chunk(4 + blk)
                    v_block(range(4 * blk, 4 * blk + 4))

            # ---------------- stage 2+3: attention, gather, out proj ----
            with (
                tc.tile_pool(name="pt", bufs=36) as ptp,
                tc.tile_pool(name="recip", bufs=4) as recipp,
                tc.tile_pool(name="bc", bufs=3) as bcp,
                tc.tile_pool(name="atv", bufs=3) as atvp,
                tc.tile_pool(name="w2", bufs=1) as w2p,
                tc.tile_pool(name="agr", bufs=3) as agrp,
                tc.tile_pool(name="outsb", bufs=3) as outsbp,
                tc.tile_pool(name="ps_s", bufs=4, space="PSUM") as pss,
                tc.tile_pool(name="ps_a", bufs=2, space="PSUM") as psa,
                tc.tile_pool(name="ps_o", bufs=2, space="PSUM") as pso,
            ):
                w2sb = w2p.tile([P, KC * CPC], BF16, tag="w2")
                nc.sync.dma_start(
                    w2sb[:].rearrange("p (c n) -> p c n", n=CPC),
                    wout.ap().rearrange("(c p) n -> p c n", p=P),
                )
                w23 = w2sb[:].rearrange("p (c n) -> p c n", n=CPC)

                def s_pass(qb, h):
                    """score matmuls + exp (+causal mask) for one head/qblock.
                    Diagonal k-tiles first so their exp+mask (on the PV
                    critical path) complete while off-diagonal scores stream.
                    """
                    qt = yts[h // 2]
                    kt_c = yts[4 + h // 2]
                    poff = (h % 2) * 64
                    nkt = 4 * qb + 4
                    kts = list(range(4 * qb, nkt)) + list(range(0, 4 * qb))
                    pts = []
                    for kt in kts:
                        j = kt - 4 * qb  # >=0 on diagonal tiles
                        qoff = max(j, 0) * P
                        ps = pss.tile([P, QB], F32, tag="ps")
                        nc.tensor.matmul(
                            ps[:, qoff:QB],
                            kt_c[poff:poff + 64, kt * P:(kt + 1) * P],
                            qt[poff:poff + 64, qb * QB + qoff:(qb + 1) * QB],
                            start=True, stop=True,
                        )
                        pt = ptp.tile([P, QB], BF16, tag="pt")
                        nc.scalar.activation(
                            pt[:, qoff:QB], ps[:, qoff:QB], EXP, scale=SCALE
                        )
                        if j >= 0:
                            # zero where q < k (also fills the stale prefix)
                            nc.gpsimd.affine_select(
                                out=pt[:],
                                in_=pt[:],
                                compare_op=mybir.AluOpType.is_ge,
                                fill=0.0,
                                base=-j * P,
                                pattern=[[1, QB]],
                                channel_multiplier=-1,
                            )
                        pts.append((kt, pt))
                    return pts

                def pv_pass(qb, h, pts, ag_in, row):
                    pa = psa.tile([P, QB], F32, tag="pa")
                    for i, (kt, pt) in enumerate(pts):
                        nc.tensor.matmul(
                            pa[0:65, :],
                            vaug3s[h][:, kt, :],
                            pt[:],
                            start=(i == 0),
                            stop=(i == len(pts) - 1),
                        )
                    sums = recipp.tile([1, QB], F32, tag="sums")
                    nc.vector.tensor_copy(sums[:], pa[64:65, :])
                    recip = recipp.tile([1, QB], F32, tag="recip")
                    nc.vector.reciprocal_approx_fast(recip[:], sums[:])
                    bc = bcp.tile([64, QB], F32, tag="bc")
                    nc.gpsimd.partition_broadcast(bc[:], recip[:])
                    atv = atvp.tile([64, QB], BF16, tag="atv")
                    nc.vector.tensor_mul(atv[:], pa[0:64, :], bc[:])
                    nc.sync.dma_start(
                        ag_in[row * 64:(row + 1) * 64, :], atv[:]
                    )

                def gather(ag_in, ag_out):
                    nc.gpsimd.collective_compute(
                        "AllGather",
                        mybir.AluOpType.bypass,
                        replica_groups=groups,
                        ins=[ag_in.opt()],
                        outs=[ag_out.opt()],
                    )

                def out_proj(qb, ag_outs):
                    # w_out rows are host-permuted to match the gathered
                    # row order [even0-3, odd0-3, even4-5, odd4-5, ...]
                    agr3s = []
                    for gi, ago in enumerate(ag_outs):
                        ncch = 2 * (GGRP[gi][1] - GGRP[gi][0]) * 64 // P
                        agr = agrp.tile(
                            [P, ncch * QB], BF16,
                            name=f"agr{qb}_{gi}", tag=f"agr{gi}",
                        )
                        nc.sync.dma_start(
                            agr[:].rearrange("p (c n) -> p c n", n=QB),
                            ago[:].rearrange("(c p) n -> p c n", p=P),
                        )
                        agr3s.append(
                            agr[:].rearrange("p (c n) -> p c n", n=QB)
                        )
                    # chunk cc -> (gather buffer, sub-chunk)
                    ccmap = [(0, 0), (0, 1), (0, 2), (0, 3),
                             (1, 0), (1, 1), (2, 0), (2, 1)]
                    for oc in range(4):
                        po = pso.tile([P, QB], F32, tag="po")
                        for cc in range(KC):
                            gi, sub = ccmap[cc]
                            nc.tensor.matmul(
                                po[:],
                                w23[:, cc, oc * P:(oc + 1) * P],
                                agr3s[gi][:, sub, :],
                                start=(cc == 0),
                                stop=(cc == KC - 1),
                            )
                        osb = outsbp.tile([P, QB], F32, tag="osb")
                        nc.vector.tensor_scalar_add(
                            osb[:], po[:], bo_sb[:, oc:oc + 1]
                        )
                        nc.sync.dma_start(
                            outT[oc * P:(oc + 1) * P, qb * QB:(qb + 1) * QB],
                            osb[:],
                        )

                # gather groups: heads 0-3, heads 4-5, heads 6-7
                GGRP = [(0, 4), (4, 6), (6, 8)]

                pending_outproj = None
                for qb in range(NQB):
                    ag_ins = [
                        dramp.tile(
                            [(e - s) * 64, QB], BF16,
                            name=f"agin{qb}_{i}", tag=f"agin{qb}_{i}",
                        )
                        for i, (s, e) in enumerate(GGRP)
                    ]
                    ag_outs = [
                        dramp.tile(
                            [2 * (e - s) * 64, QB], BF16,
                            name=f"agout{qb}_{i}", tag=f"agout{qb}_{i}",
                        )
                        for i, (s, e) in enumerate(GGRP)
                    ]
                    grp_of = {}
                    for i, (s, e) in enumerate(GGRP):
                        for h in range(s, e):
                            grp_of[h] = (i, h - s)
                    prev = None
                    for h in range(HPC):
                        cur = s_pass(qb, h)
                        if h == 3 and pending_outproj is not None:
                            # previous qblock's out-projection: its gather
                            # waits hide behind this qblock's score matmuls
                            pending_outproj()
                            pending_outproj = None
                        if prev is not None:
                            hp = h - 1
                            gi, row = grp_of[hp]
                            pv_pass(qb, hp, prev, ag_ins[gi], row)
                            if hp in (3, 5):
                                gather(ag_ins[gi], ag_outs[gi])
                        prev = cur
                    gi, row = grp_of[HPC - 1]
                    pv_pass(qb, HPC - 1, prev, ag_ins[gi], row)
                    gather(ag_ins[gi], ag_outs[gi])
                    pending_outproj = (
                        lambda qb=qb, ag_outs=ag_outs: out_proj(qb, ag_outs)
                    )
                pending_outproj()

    nc.compile()
    return nc


def kernel(x, w_qkv, b_qkv, w_out, b_out):
    x = np.asarray(x, dtype=np.float32)
    w_qkv = np.asarray(w_qkv, dtype=np.float32)
    b_qkv = np.asarray(b_qkv, dtype=np.float32)
    w_out = np.asarray(w_out, dtype=np.float32)
    b_out = np.asarray(b_out, dtype=np.float32)

    if "nc" not in _CACHE:
        _CACHE["nc"] = build()
    nc = _CACHE["nc"]

    # V bias passes through softmax unchanged; fold it into the out bias
    bv_perm_all = b_qkv[2 * C:3 * C]

    in_maps = []
    for c in range(NCORES):
        b = c // 2
        h0 = (c % 2) * HPC
        cols = slice(h0 * D, h0 * D + CPC)
        wq = np.concatenate(
            [w_qkv[:, cols], w_qkv[:, C:][:, cols], w_qkv[:, 2 * C:][:, cols]],
            axis=1,
        )
        bq = np.concatenate(
            [b_qkv[cols], b_qkv[C:][cols], b_qkv[2 * C:][cols]]
        ).reshape(1, 3 * CPC)
        half = slice((c % 2) * CPC, (c % 2) * CPC + CPC)
        wo = w_out[:, half]
        # rows permuted to the gathered channel order:
        # [even h0-3, odd h0-3, even h4-5, odd h4-5, even h6-7, odd h6-7]
        wo_perm = np.concatenate(
            [wo[0:256], wo[512:768],
             wo[256:384], wo[768:896],
             wo[384:512], wo[896:1024]], axis=0
        )
        bout_eff = b_out[half] + bv_perm_all @ w_out[:, half]
        in_maps.append({
            "xT": np.ascontiguousarray(x[b].T.astype(ml_dtypes.bfloat16)),
            "wqkv": np.ascontiguousarray(wq.astype(ml_dtypes.bfloat16)),
            "bqkv": np.ascontiguousarray(bq),
            "wout": np.ascontiguousarray(wo_perm.astype(ml_dtypes.bfloat16)),
            "bout": np.ascontiguousarray(bout_eff).reshape(1, CPC),
        })

    kwargs = {}
    tdir = os.environ.get("KERNEL_TRACE_DIR")
    if tdir:
        kwargs = dict(trace=True, tmpdir=tdir)
    res = run_bass_kernel_spmd(
        nc, in_maps, core_ids=list(range(NCORES)), **kwargs
    )
    _CACHE["last_results"] = res

    out = np.empty((B, T, C), dtype=np.float32)
    for c in range(NCORES):
        b = c // 2
        half = slice((c % 2) * CPC, (c % 2) * CPC + CPC)
        out[b][:, half] = res.results[c]["outT"].T
    return out



# revision 7
# speedup vs baseline: 1.0389x; 1.0389x over previous
"""Masked multi-head self-attention kernel for 8 Trainium2 NeuronCores.

Full module: qkv projection -> causal softmax attention (16 heads) -> out
projection, for x[4, 2048, 1024].

Sharding: core c handles batch b = c//2 and heads h0 = (c%2)*8 .. h0+8.
QKV projection + attention are fully local to a core.  The out projection
contracts over all 16 heads' channels, so the two cores of a batch exchange
their attention outputs with pairwise AllGathers (chunked over heads and
query blocks for overlap) and each computes half of the output columns.
Each core returns out[b][:, half].T (transposed: [512, 2048]); the host
reassembles.  Inputs are re-laid-out per core on the host: x transposed,
qkv weight columns / out-proj rows sliced and permuted to the gather order,
biases pre-shuffled into per-partition layout.
"""

import math
import os
import sys

for _p in ("/opt/trn_rl_repo", "/root/.axon_site/_ro/trn_rl_repo"):
    if os.path.isdir(_p) and _p not in sys.path:
        sys.path.insert(0, _p)
        break

import ml_dtypes
import numpy as np

import concourse.bass as bass
import concourse.mybir as mybir
import concourse.tile as tile
from concourse import bacc
from concourse.bass_utils import run_bass_kernel_spmd

B, T, C, H = 4, 2048, 1024, 16
D = 64                 # head dim
NCORES = 8
HPC = H // 2           # heads per core = 8
CPC = HPC * D          # channels per core = 512
P = 128                # partitions
QB = 512               # query block
NQB = T // QB          # 4
KC = C // P            # contraction chunks for C = 8
NTT = T // P           # 16 t-tiles
SCALE = 1.0 / math.sqrt(D)

F32 = mybir.dt.float32
BF16 = mybir.dt.bfloat16
EXP = mybir.ActivationFunctionType.Exp
IDENT = mybir.ActivationFunctionType.Identity

_CACHE = {}


def build():
    nc = bacc.Bacc("TRN2", num_devices=NCORES, debug=False)

    xT = nc.dram_tensor("xT", [C, T], BF16, kind="ExternalInput")
    wqkv = nc.dram_tensor("wqkv", [C, 3 * CPC], BF16, kind="ExternalInput")
    bqkv = nc.dram_tensor("bqkv", [P, 12], F32, kind="ExternalInput")
    wout = nc.dram_tensor("wout", [C, CPC], BF16, kind="ExternalInput")
    bout = nc.dram_tensor("bout", [P, 4], F32, kind="ExternalInput")
    outT = nc.dram_tensor("outT", [CPC, T], F32, kind="ExternalOutput")

    groups = [[0, 1], [2, 3], [4, 5], [6, 7]]

    with tile.TileContext(nc) as tc:
        with (
            tc.tile_pool(name="const", bufs=1) as constp,
            tc.tile_pool(name="ytp", bufs=1) as ytp,
            tc.tile_pool(name="vaugp", bufs=1) as vaugp,
            tc.tile_pool(name="dram", bufs=1, space="DRAM") as dramp,
        ):
            # biases arrive pre-shuffled: bq_sb[p, n] = bias[n*128 + p]
            bq_sb = constp.tile([P, 12], F32, tag="bq")
            nc.sync.dma_start(bq_sb[:], bqkv.ap())
            bo_sb = constp.tile([P, 4], F32, tag="bo")
            nc.sync.dma_start(bo_sb[:], bout.ap())

            # Q^T,K^T: 8 chunks of [128 ch, 2048 t] (Q: 0-3, K: 4-7)
            yts = [
                ytp.tile([P, T], BF16, name=f"yt{n}", tag=f"yt{n}")
                for n in range(8)
            ]
            # V natural (+ones col), all heads in one tile:
            # vaug4[p, h, kt, c]; c=64 is the ones column
            vaug = vaugp.tile([P, HPC * NTT * 65], BF16, tag="vaug")
            vaug4 = vaug[:].rearrange("p (h k c) -> p h k c", h=HPC, c=65)
            nc.vector.memset(vaug4[:, :, :, 64:65], 1.0)

            # ---------------- stage 1: qkv projection, V ----------------
            with (
                tc.tile_pool(name="xtp", bufs=1) as xtp,
                tc.tile_pool(name="wqkvp", bufs=1) as wqkvp,
                tc.tile_pool(name="ps_y", bufs=2, space="PSUM") as psy,
                tc.tile_pool(name="ps_v", bufs=2, space="PSUM") as psv,
            ):
                # x^T chunks resident in SBUF: [128 ch, 2048 t] each
                # (issued on sync queue; weights go on the scalar queue so
                # the two streams load in parallel)
                xts = [
                    xtp.tile([P, T], BF16, name=f"xt{cc}", tag=f"xt{cc}")
                    for cc in range(KC)
                ]
                for cc in range(KC):
                    nc.sync.dma_start(
                        xts[cc][:], xT[cc * P:(cc + 1) * P, :]
                    )
                # all qkv weights in one tile: w3[p, kc, 1536]
                # (cols 0-1023: q|k chunks n=0..7; 1024-1535: v)
                wq_sb = wqkvp.tile([P, KC * 3 * CPC], BF16, tag="wq")
                w3 = wq_sb[:].rearrange("p (c n) -> p c n", n=3 * CPC)
                for cc in range(KC):
                    nc.scalar.dma_start(
                        w3[:, cc, :], wqkv[cc * P:(cc + 1) * P, :]
                    )

                def qk_chunk(n):
                    # halves of 2 query-blocks each; one PSUM pair per half
                    # so the first half's drain overlaps the second half's
                    # matmuls and the next chunk never stalls on a bank.
                    for half in range(2):
                        py = psy.tile([P, 2 * QB], F32, tag="py")
                        for kc in range(KC):
                            for i in range(2):
                                tc4 = 2 * half + i
                                nc.tensor.matmul(
                                    py[:, i * QB:(i + 1) * QB],
                                    w3[:, kc, n * P:(n + 1) * P],
                                    xts[kc][:, tc4 * QB:(tc4 + 1) * QB],
                                    start=(kc == 0),
                                    stop=(kc == KC - 1),
                                )
                        # bias-add on the (otherwise idle) scalar engine
                        nc.scalar.activation(
                            yts[n][:, half * 2 * QB:(half + 1) * 2 * QB],
                            py[:],
                            IDENT,
                            bias=bq_sb[:, n:n + 1],
                        )

                def v_block(tt0):
                    # V natural for t-tiles (tt0, tt0+1): x^T stationary;
                    # V bias is folded into the output bias on the host
                    pv = psv.tile([P, 2 * QB], F32, tag="pv")
                    for i in range(2):
                        tt = tt0 + i
                        for kc in range(KC):
                            nc.tensor.matmul(
                                pv[:, i * QB:(i + 1) * QB],
                                xts[kc][:, tt * P:(tt + 1) * P],
                                w3[:, kc, 2 * CPC:3 * CPC],
                                start=(kc == 0),
                                stop=(kc == KC - 1),
                            )
                    # one strided copy distributes both t-tiles to all heads
                    nc.vector.tensor_copy(
                        vaug4[:, :, tt0:tt0 + 2, 0:64],
                        pv[:].rearrange("p (t h d) -> p h t d", t=2, d=64),
                    )

                for blk in range(4):
                    qk_chunk(blk)
                    qk_chunk(4 + blk)
                    v_block(4 * blk)
                    v_block(4 * blk + 2)

            # ---------------- stage 2+3: attention, gather, out proj ----
            with (
                tc.tile_pool(name="pt", bufs=12) as ptp,
                tc.tile_pool(name="recip", bufs=4) as recipp,
                tc.tile_pool(name="bc", bufs=3) as bcp,
                tc.tile_pool(name="atv", bufs=3) as atvp,
                tc.tile_pool(name="w2", bufs=1) as w2p,
                tc.tile_pool(name="agr", bufs=3) as agrp,
                tc.tile_pool(name="outsb", bufs=3) as outsbp,
                tc.tile_pool(name="ps_s", bufs=2, space="PSUM") as pss,
                tc.tile_pool(name="ps_a", bufs=2, space="PSUM") as psa,
                tc.tile_pool(name="ps_o", bufs=2, space="PSUM") as pso,
            ):
                w2sb = w2p.tile([P, KC * CPC], BF16, tag="w2")
                nc.scalar.dma_start(
                    w2sb[:].rearrange("p (c n) -> p c n", n=CPC),
                    wout.ap().rearrange("(c p) n -> p c n", p=P),
                )
                w23 = w2sb[:].rearrange("p (c n) -> p c n", n=CPC)

                def s_pass(qb, h):
                    """Scores + exp (+causal mask) for one head/qblock.

                    k-tiles processed in pairs sharing a 2-bank PSUM tile so
                    off-diagonal exp runs as one [128,1024] activation
                    (amortizes the fixed activation overhead).  Diagonal
                    pairs first so their exp+mask (on the PV critical path)
                    complete while off-diagonal scores stream.
                    """
                    qt = yts[h // 2]
                    kt_c = yts[4 + h // 2]
                    poff = (h % 2) * 64
                    kts = list(range(4 * qb, 4 * qb + 4)) + list(range(0, 4 * qb))
                    out = []
                    for pi in range(len(kts) // 2):
                        k0, k1 = kts[2 * pi], kts[2 * pi + 1]
                        diag = pi < 2
                        ps = pss.tile([P, 2 * QB], F32, tag="ps")
                        pt = ptp.tile([P, 2 * QB], BF16, tag="pt")
                        qoffs = []
                        for i, kt in enumerate((k0, k1)):
                            j = kt - 4 * qb
                            qoff = j * P if diag else 0
                            qoffs.append(qoff)
                            nc.tensor.matmul(
                                ps[:, i * QB + qoff:(i + 1) * QB],
                                kt_c[poff:poff + 64, kt * P:(kt + 1) * P],
                                qt[poff:poff + 64,
                                   qb * QB + qoff:(qb + 1) * QB],
                                start=True, stop=True,
                            )
                        if diag:
                            for i in range(2):
                                qoff = qoffs[i]
                                nc.scalar.activation(
                                    pt[:, i * QB + qoff:(i + 1) * QB],
                                    ps[:, i * QB + qoff:(i + 1) * QB],
                                    EXP, scale=SCALE,
                                )
                                # zero the triangle where q < k in the
                                # 128-col diagonal block
                                nc.gpsimd.affine_select(
                                    out=pt[:, i * QB + qoff:i * QB + qoff + P],
                                    in_=pt[:, i * QB + qoff:i * QB + qoff + P],
                                    compare_op=mybir.AluOpType.is_ge,
                                    fill=0.0,
                                    base=0,
                                    pattern=[[1, P]],
                                    channel_multiplier=-1,
                                )
                        else:
                            nc.scalar.activation(
                                pt[:], ps[:], EXP, scale=SCALE
                            )
                        out.append((k0, k1, pt, qoffs))
                    return out

                def pv_pass(qb, h, pairs, ag_in, row):
                    pa = psa.tile([P, QB], F32, tag="pa")
                    nmm = 4 * qb + 4
                    mi = 0
                    for (k0, k1, pt, qoffs) in pairs:
                        for i, kt in enumerate((k0, k1)):
                            qoff = qoffs[i]
                            nc.tensor.matmul(
                                pa[0:65, qoff:QB],
                                vaug4[:, h, kt, :],
                                pt[:, i * QB + qoff:(i + 1) * QB],
                                start=(mi == 0),
                                stop=(mi == nmm - 1),
                                skip_group_check=True,
                            )
                            mi += 1
                    sums = recipp.tile([1, QB], F32, tag="sums")
                    nc.vector.tensor_copy(sums[:], pa[64:65, :])
                    recip = recipp.tile([1, QB], F32, tag="recip")
                    nc.vector.reciprocal_approx_fast(recip[:], sums[:])
                    bc = bcp.tile([64, QB], F32, tag="bc")
                    nc.gpsimd.partition_broadcast(bc[:], recip[:])
                    atv = atvp.tile([64, QB], BF16, tag="atv")
                    nc.vector.tensor_mul(atv[:], pa[0:64, :], bc[:])
                    nc.sync.dma_start(
                        ag_in[row * 64:(row + 1) * 64, :], atv[:]
                    )

                def gather(ag_in, ag_out):
                    nc.gpsimd.collective_compute(
                        "AllGather",
                        mybir.AluOpType.bypass,
                        replica_groups=groups,
                        ins=[ag_in.opt()],
                        outs=[ag_out.opt()],
                    )

                # gather groups: heads 0-3, heads 4-5, heads 6-7
                GGRP = [(0, 4), (4, 6), (6, 8)]
                # chunk cc -> (gather buffer, sub-chunk)
                CCMAP = [(0, 0), (0, 1), (0, 2), (0, 3),
                         (1, 0), (1, 1), (2, 0), (2, 1)]

                def load_agr(qb, ag_outs, gi):
                    # w_out rows are host-permuted to match the gathered
                    # row order [even0-3, odd0-3, even4-5, odd4-5, ...]
                    # issued on the scalar queue: the sync queue carries the
                    # atv DMAs that feed the gathers these loads wait on.
                    ago = ag_outs[gi]
                    ncch = 2 * (GGRP[gi][1] - GGRP[gi][0]) * 64 // P
                    agr = agrp.tile(
                        [P, ncch * QB], BF16,
                        name=f"agr{qb}_{gi}", tag=f"agr{gi}",
                    )
                    nc.scalar.dma_start(
                        agr[:].rearrange("p (c n) -> p c n", n=QB),
                        ago[:].rearrange("(c p) n -> p c n", p=P),
                    )
                    return agr[:].rearrange("p (c n) -> p c n", n=QB)

                def out_proj(qb, agr3s):
                    for oc in range(4):
                        po = pso.tile([P, QB], F32, tag="po")
                        for cc in range(KC):
                            gi, sub = CCMAP[cc]
                            nc.tensor.matmul(
                                po[:],
                                w23[:, cc, oc * P:(oc + 1) * P],
                                agr3s[gi][:, sub, :],
                                start=(cc == 0),
                                stop=(cc == KC - 1),
                            )
                        osb = outsbp.tile([P, QB], F32, tag="osb")
                        nc.vector.tensor_scalar_add(
                            osb[:], po[:], bo_sb[:, oc:oc + 1]
                        )
                        nc.sync.dma_start(
                            outT[oc * P:(oc + 1) * P, qb * QB:(qb + 1) * QB],
                            osb[:],
                        )

                def make_bufs(qb):
                    ag_ins = [
                        dramp.tile(
                            [(e - s) * 64, QB], BF16,
                            name=f"agin{qb}_{i}", tag=f"agin{qb}_{i}",
                        )
                        for i, (s, e) in enumerate(GGRP)
                    ]
                    ag_outs = [
                        dramp.tile(
                            [2 * (e - s) * 64, QB], BF16,
                            name=f"agout{qb}_{i}", tag=f"agout{qb}_{i}",
                        )
                        for i, (s, e) in enumerate(GGRP)
                    ]
                    return ag_ins, ag_outs

                grp_of = {}
                for i, (s, e) in enumerate(GGRP):
                    for h in range(s, e):
                        grp_of[h] = (i, h - s)
                # last head of each group, per group index
                last_of_grp = {i: e - 1 for i, (s, e) in enumerate(GGRP)}

                # pending: (qb, ag_outs) of the previous query block whose
                # out-projection still has to be emitted
                pending = None
                for qb in range(NQB - 1):
                    ag_ins, ag_outs = make_bufs(qb)
                    prev = None
                    agr3s = None
                    for h in range(HPC):
                        cur = s_pass(qb, h)
                        if prev is not None:
                            hp = h - 1
                            gi, row = grp_of[hp]
                            pv_pass(qb, hp, prev, ag_ins[gi], row)
                            if hp == last_of_grp[gi]:
                                gather(ag_ins[gi], ag_outs[gi])
                        if pending is not None:
                            # previous qblock's out-projection, staggered:
                            # agr loads early (each gather is done well
                            # before its load is issued), matmuls at h==5
                            # so they never head-block the tensor queue
                            pqb, pago = pending
                            if h == 3:
                                agr3s = [load_agr(pqb, pago, 0),
                                         load_agr(pqb, pago, 1)]
                            elif h == 4:
                                agr3s.append(load_agr(pqb, pago, 2))
                            elif h == 5:
                                out_proj(pqb, agr3s)
                                pending = None
                        prev = cur
                    gi, row = grp_of[HPC - 1]
                    pv_pass(qb, HPC - 1, prev, ag_ins[gi], row)
                    gather(ag_ins[gi], ag_outs[gi])
                    pending = (qb, ag_outs)

                # ---- last query block: heads 6,7 first so their gather
                # (the out-projection's last dependency) launches early;
                # out-projection is emitted group-major into 4 live PSUM
                # halves so only the final group's 8 matmuls sit behind
                # the final gather.
                qb = NQB - 1
                ag_ins, ag_outs = make_bufs(qb)
                horder = [6, 7, 0, 1, 2, 3, 4, 5]
                pairs_of = {}
                emitted_pv = set()

                def emit_pv(h):
                    gi, row = grp_of[h]
                    pv_pass(qb, h, pairs_of.pop(h), ag_ins[gi], row)
                    emitted_pv.add(h)
                    s, e = GGRP[gi]
                    if all(x in emitted_pv for x in range(s, e)):
                        gather(ag_ins[gi], ag_outs[gi])

                lagr = {}
                for idx, h in enumerate(horder):
                    pairs_of[h] = s_pass(qb, h)
                    if idx >= 1:
                        emit_pv(horder[idx - 1])
                    if pending is not None:
                        pqb, pago = pending
                        if idx == 2:
                            lagr[0] = load_agr(pqb, pago, 0)
                            lagr[1] = load_agr(pqb, pago, 1)
                        elif idx == 3:
                            lagr[2] = load_agr(pqb, pago, 2)
                        elif idx == 4:
                            out_proj(pqb, [lagr[0], lagr[1], lagr[2]])
                            pending = None

                # group-major out-projection for the last block: g2 first
                # (its gather launched earliest), g1 last — only its 8
                # matmuls sit behind the final gather.
                agr3s = {}
                po_pairs = None

                def po_half(oc):
                    return po_pairs[oc // 2][:, (oc % 2) * QB:(oc % 2 + 1) * QB]

                def oproj_group(gi, first, last):
                    chunks = [cc for cc in range(KC) if CCMAP[cc][0] == gi]
                    for oc in range(4):
                        for k, cc in enumerate(chunks):
                            nc.tensor.matmul(
                                po_half(oc),
                                w23[:, cc, oc * P:(oc + 1) * P],
                                agr3s[gi][:, CCMAP[cc][1], :],
                                start=(first and k == 0),
                                stop=(last and k == len(chunks) - 1),
                                skip_group_check=True,
                            )

                # in-loop emissions ended with s_pass(5) then pv(4);
                # g2 (heads 6,7) and g0 gathers are already in flight
                agr3s[2] = load_agr(qb, ag_outs, 2)
                agr3s[0] = load_agr(qb, ag_outs, 0)
                po_pairs = [
                    pss.tile([P, 2 * QB], F32, tag="ps", name=f"po{i}")
                    for i in range(2)
                ]
                oproj_group(2, True, False)
                emit_pv(5)  # last pv + gather g1
                oproj_group(0, False, False)
                agr3s[1] = load_agr(qb, ag_outs, 1)
                oproj_group(1, False, True)
                for oc in range(4):
                    osb = outsbp.tile([P, QB], F32, tag="osb")
                    nc.vector.tensor_scalar_add(
                        osb[:], po_half(oc), bo_sb[:, oc:oc + 1]
                    )
                    nc.sync.dma_start(
                        outT[oc * P:(oc + 1) * P, qb * QB:(qb + 1) * QB],
                        osb[:],
                    )

    nc.compile()
    return nc


def kernel(x, w_qkv, b_qkv, w_out, b_out):
    x = np.asarray(x, dtype=np.float32)
    w_qkv = np.asarray(w_qkv, dtype=np.float32)
    b_qkv = np.asarray(b_qkv, dtype=np.float32)
    w_out = np.asarray(w_out, dtype=np.float32)
    b_out = np.asarray(b_out, dtype=np.float32)

    if "nc" not in _CACHE:
        _CACHE["nc"] = build()
    nc = _CACHE["nc"]

    # V bias passes through softmax unchanged; fold it into the out bias
    bv_all = b_qkv[2 * C:3 * C]

    in_maps = []
    for c in range(NCORES):
        b = c // 2
        h0 = (c % 2) * HPC
        cols = slice(h0 * D, h0 * D + CPC)
        wq = np.concatenate(
            [w_qkv[:, cols], w_qkv[:, C:][:, cols], w_qkv[:, 2 * C:][:, cols]],
            axis=1,
        )
        bq = np.concatenate(
            [b_qkv[cols], b_qkv[C:][cols], b_qkv[2 * C:][cols]]
        )
        # per-partition layout: bq_sb[p, n] = bq[n*128 + p]
        bq_pp = np.ascontiguousarray(bq.reshape(12, P).T)
        half = slice((c % 2) * CPC, (c % 2) * CPC + CPC)
        wo = w_out[:, half]
        # rows permuted to the gathered channel order:
        # [even h0-3, odd h0-3, even h4-5, odd h4-5, even h6-7, odd h6-7]
        wo_perm = np.concatenate(
            [wo[0:256], wo[512:768],
             wo[256:384], wo[768:896],
             wo[384:512], wo[896:1024]], axis=0
        )
        bout_eff = b_out[half] + bv_all @ w_out[:, half]
        bo_pp = np.ascontiguousarray(bout_eff.reshape(4, P).T)
        in_maps.append({
            "xT": np.ascontiguousarray(x[b].T.astype(ml_dtypes.bfloat16)),
            "wqkv": np.ascontiguousarray(wq.astype(ml_dtypes.bfloat16)),
            "bqkv": bq_pp,
            "wout": np.ascontiguousarray(wo_perm.astype(ml_dtypes.bfloat16)),
            "bout": bo_pp,
        })

    kwargs = {}
    tdir = os.environ.get("KERNEL_TRACE_DIR")
    if tdir:
        kwargs = dict(trace=True, tmpdir=tdir)
    res = run_bass_kernel_spmd(
        nc, in_maps, core_ids=list(range(NCORES)), **kwargs
    )
    _CACHE["last_results"] = res

    out = np.empty((B, T, C), dtype=np.float32)
    for c in range(NCORES):
        b = c // 2
        half = slice((c % 2) * CPC, (c % 2) * CPC + CPC)
        out[b][:, half] = res.results[c]["outT"].T
    return out


# revision 12
# speedup vs baseline: 1.0578x; 1.0183x over previous
"""Masked multi-head self-attention kernel for 8 Trainium2 NeuronCores.

Full module: qkv projection -> causal softmax attention (16 heads) -> out
projection, for x[4, 2048, 1024].

Sharding: core c handles batch b = c//2 and heads h0 = (c%2)*8 .. h0+8.
QKV projection + attention are fully local to a core.  The out projection
contracts over all 16 heads' channels, so the two cores of a batch exchange
their attention outputs with pairwise AllGathers (chunked over heads and
query blocks for overlap) and each computes half of the output columns.
Each core returns out[b][:, half].T (transposed: [512, 2048], bf16); the
host reassembles.  Inputs are re-laid-out per core on the host: x
transposed, qkv weight columns / out-proj rows sliced and permuted to the
gather order, biases pre-shuffled into per-partition layout.

Attention is emitted as a software pipeline over units (qb, h): at step i
the score pass of unit i runs while the PV pass of unit i-1 and the
softmax-normalization of unit i-2 drain, keeping every engine queue free
of head-of-line waits.  qb0 and qb1 are interleaved head-by-head: qb0
alone has too little tensor work to cover its normalization chains.
"""

import math
import os
import sys

for _p in ("/opt/trn_rl_repo", "/root/.axon_site/_ro/trn_rl_repo"):
    if os.path.isdir(_p) and _p not in sys.path:
        sys.path.insert(0, _p)
        break

import ml_dtypes
import numpy as np

import concourse.bass as bass
import concourse.mybir as mybir
import concourse.tile as tile
from concourse import bacc
from concourse.bass_utils import run_bass_kernel_spmd

B, T, C, H = 4, 2048, 1024, 16
D = 64                 # head dim
NCORES = 8
HPC = H // 2           # heads per core = 8
CPC = HPC * D          # channels per core = 512
P = 128                # partitions
QB = 512               # query block
NQB = T // QB          # 4
KC = C // P            # contraction chunks for C = 8
NTT = T // P           # 16 t-tiles
SCALE = 1.0 / math.sqrt(D)

F32 = mybir.dt.float32
BF16 = mybir.dt.bfloat16
EXP = mybir.ActivationFunctionType.Exp
IDENT = mybir.ActivationFunctionType.Identity

_CACHE = {}


def build():
    nc = bacc.Bacc("TRN2", num_devices=NCORES, debug=False)

    xT = nc.dram_tensor("xT", [C, T], BF16, kind="ExternalInput")
    wqkv = nc.dram_tensor("wqkv", [C, 3 * CPC], BF16, kind="ExternalInput")
    bqkv = nc.dram_tensor("bqkv", [P, 12], F32, kind="ExternalInput")
    wout = nc.dram_tensor("wout", [C, CPC], BF16, kind="ExternalInput")
    bout = nc.dram_tensor("bout", [P, 4], F32, kind="ExternalInput")
    outT = nc.dram_tensor("outT", [CPC, T], BF16, kind="ExternalOutput")

    groups = [[0, 1], [2, 3], [4, 5], [6, 7]]

    with tile.TileContext(nc) as tc:
        with (
            tc.tile_pool(name="const", bufs=1) as constp,
            tc.tile_pool(name="ytp", bufs=1) as ytp,
            tc.tile_pool(name="vaugp", bufs=1) as vaugp,
            tc.tile_pool(name="dram", bufs=1, space="DRAM") as dramp,
        ):
            # biases arrive pre-shuffled: bq_sb[p, n] = bias[n*128 + p]
            bq_sb = constp.tile([P, 12], F32, tag="bq")
            nc.sync.dma_start(bq_sb[:], bqkv.ap())
            bo_sb = constp.tile([P, 4], F32, tag="bo")
            nc.sync.dma_start(bo_sb[:], bout.ap())

            # Q^T,K^T: 8 chunks of [128 ch, 2048 t] (Q: 0-3, K: 4-7)
            yts = [
                ytp.tile([P, T], BF16, name=f"yt{n}", tag=f"yt{n}")
                for n in range(8)
            ]
            # V natural (+ones col), all heads in one tile:
            # vaug4[p, h, kt, c]; c=64 is the ones column
            vaug = vaugp.tile([P, HPC * NTT * 65], BF16, tag="vaug")
            vaug4 = vaug[:].rearrange("p (h k c) -> p h k c", h=HPC, c=65)
            nc.vector.memset(vaug4[:, :, :, 64:65], 1.0)

            # ---------------- stage 1: qkv projection, V ----------------
            with (
                tc.tile_pool(name="xtp", bufs=1) as xtp,
                tc.tile_pool(name="wqkvp", bufs=1) as wqkvp,
                tc.tile_pool(name="ps_y", bufs=2, space="PSUM") as psy,
                tc.tile_pool(name="ps_v", bufs=2, space="PSUM") as psv,
            ):
                # x^T chunks resident in SBUF: [128 ch, 2048 t] each
                # (sync queue; weights go on the scalar queue so the two
                # streams load in parallel).  Chunk 0 is split so the very
                # first matmul's data lands as early as possible.
                xts = [
                    xtp.tile([P, T], BF16, name=f"xt{cc}", tag=f"xt{cc}")
                    for cc in range(KC)
                ]
                nc.sync.dma_start(xts[0][:, 0:T // 2], xT[0:P, 0:T // 2])
                nc.sync.dma_start(xts[0][:, T // 2:T], xT[0:P, T // 2:T])
                for cc in range(1, KC):
                    nc.sync.dma_start(
                        xts[cc][:], xT[cc * P:(cc + 1) * P, :]
                    )
                # all qkv weights in one tile: w3[p, kc, 1536]
                # (cols 0-1023: q|k chunks n=0..7; 1024-1535: v)
                wq_sb = wqkvp.tile([P, KC * 3 * CPC], BF16, tag="wq")
                w3 = wq_sb[:].rearrange("p (c n) -> p c n", n=3 * CPC)
                for cc in range(KC):
                    nc.scalar.dma_start(
                        w3[:, cc, :], wqkv[cc * P:(cc + 1) * P, :]
                    )

                def qk_chunk(n):
                    # halves of 2 query-blocks each; one PSUM pair per half
                    # so the first half's drain overlaps the second half's
                    # matmuls and the next chunk never stalls on a bank.
                    for half in range(2):
                        py = psy.tile([P, 2 * QB], F32, tag="py")
                        for kc in range(KC):
                            for i in range(2):
                                tc4 = 2 * half + i
                                nc.tensor.matmul(
                                    py[:, i * QB:(i + 1) * QB],
                                    w3[:, kc, n * P:(n + 1) * P],
                                    xts[kc][:, tc4 * QB:(tc4 + 1) * QB],
                                    start=(kc == 0),
                                    stop=(kc == KC - 1),
                                )
                        # bias-add on the (otherwise idle) scalar engine
                        nc.scalar.activation(
                            yts[n][:, half * 2 * QB:(half + 1) * 2 * QB],
                            py[:],
                            IDENT,
                            bias=bq_sb[:, n:n + 1],
                        )

                def v_block(tt0):
                    # V natural for t-tiles (tt0, tt0+1): x^T stationary;
                    # V bias is folded into the output bias on the host
                    pv = psv.tile([P, 2 * QB], F32, tag="pv")
                    for i in range(2):
                        tt = tt0 + i
                        for kc in range(KC):
                            nc.tensor.matmul(
                                pv[:, i * QB:(i + 1) * QB],
                                xts[kc][:, tt * P:(tt + 1) * P],
                                w3[:, kc, 2 * CPC:3 * CPC],
                                start=(kc == 0),
                                stop=(kc == KC - 1),
                            )
                    # one strided copy distributes both t-tiles to all heads
                    nc.vector.tensor_copy(
                        vaug4[:, :, tt0:tt0 + 2, 0:64],
                        pv[:].rearrange("p (t h d) -> p h t d", t=2, d=64),
                    )

                for blk in range(4):
                    qk_chunk(blk)
                    qk_chunk(4 + blk)
                    v_block(4 * blk)
                    v_block(4 * blk + 2)

            # ---------------- stage 2+3: attention, gather, out proj ----
            with (
                tc.tile_pool(name="pt", bufs=18) as ptp,
                tc.tile_pool(name="recip", bufs=4) as recipp,
                tc.tile_pool(name="bc", bufs=4) as bcp,
                tc.tile_pool(name="atv", bufs=4) as atvp,
                tc.tile_pool(name="w2", bufs=1) as w2p,
                tc.tile_pool(name="agr", bufs=3) as agrp,
                tc.tile_pool(name="outsb", bufs=3) as outsbp,
                tc.tile_pool(name="ps_s", bufs=2, space="PSUM") as pss,
                tc.tile_pool(name="ps_a", bufs=3, space="PSUM") as psa,
                tc.tile_pool(name="ps_o", bufs=1, space="PSUM") as pso,
            ):
                w2sb = w2p.tile([P, KC * CPC], BF16, tag="w2")
                nc.scalar.dma_start(
                    w2sb[:].rearrange("p (c n) -> p c n", n=CPC),
                    wout.ap().rearrange("(c p) n -> p c n", p=P),
                )
                w23 = w2sb[:].rearrange("p (c n) -> p c n", n=CPC)

                # gather groups: heads 0-3, heads 4-5, heads 6-7
                GGRP = [(0, 4), (4, 6), (6, 8)]
                # chunk cc -> (gather buffer, sub-chunk)
                CCMAP = [(0, 0), (0, 1), (0, 2), (0, 3),
                         (1, 0), (1, 1), (2, 0), (2, 1)]
                grp_of = {}
                for i, (s, e) in enumerate(GGRP):
                    for h in range(s, e):
                        grp_of[h] = (i, h - s)

                ag_ins = {}
                ag_outs = {}
                for qb in range(NQB):
                    ag_ins[qb] = [
                        dramp.tile(
                            [(e - s) * 64, QB], BF16,
                            name=f"agin{qb}_{i}", tag=f"agin{qb}_{i}",
                        )
                        for i, (s, e) in enumerate(GGRP)
                    ]
                    ag_outs[qb] = [
                        dramp.tile(
                            [2 * (e - s) * 64, QB], BF16,
                            name=f"agout{qb}_{i}", tag=f"agout{qb}_{i}",
                        )
                        for i, (s, e) in enumerate(GGRP)
                    ]

                def s_pass(qb, h):
                    """Scores + exp (+causal mask) for one head/qblock.

                    k-tiles processed in pairs sharing a 2-bank PSUM tile so
                    off-diagonal exp runs as one [128,1024] activation
                    (amortizes the fixed activation overhead).  Diagonal
                    pairs first so their exp+mask (on the PV critical path)
                    complete while off-diagonal scores stream.
                    """
                    qt = yts[h // 2]
                    kt_c = yts[4 + h // 2]
                    poff = (h % 2) * 64
                    kts = list(range(4 * qb, 4 * qb + 4)) + list(range(0, 4 * qb))
                    out = []
                    for pi in range(len(kts) // 2):
                        k0, k1 = kts[2 * pi], kts[2 * pi + 1]
                        diag = pi < 2
                        ps = pss.tile([P, 2 * QB], F32, tag="ps")
                        pt = ptp.tile([P, 2 * QB], BF16, tag="pt")
                        qoffs = []
                        for i, kt in enumerate((k0, k1)):
                            j = kt - 4 * qb
                            qoff = j * P if diag else 0
                            qoffs.append(qoff)
                            nc.tensor.matmul(
                                ps[:, i * QB + qoff:(i + 1) * QB],
                                kt_c[poff:poff + 64, kt * P:(kt + 1) * P],
                                qt[poff:poff + 64,
                                   qb * QB + qoff:(qb + 1) * QB],
                                start=True, stop=True,
                            )
                        if diag:
                            for i in range(2):
                                qoff = qoffs[i]
                                nc.scalar.activation(
                                    pt[:, i * QB + qoff:(i + 1) * QB],
                                    ps[:, i * QB + qoff:(i + 1) * QB],
                                    EXP, scale=SCALE,
                                )
                                # zero the triangle where q < k in the
                                # 128-col diagonal block
                                nc.gpsimd.affine_select(
                                    out=pt[:, i * QB + qoff:i * QB + qoff + P],
                                    in_=pt[:, i * QB + qoff:i * QB + qoff + P],
                                    compare_op=mybir.AluOpType.is_ge,
                                    fill=0.0,
                                    base=0,
                                    pattern=[[1, P]],
                                    channel_multiplier=-1,
                                )
                        else:
                            nc.scalar.activation(
                                pt[:], ps[:], EXP, scale=SCALE
                            )
                        out.append((k0, k1, pt, qoffs))
                    return out

                def pv_pass(qb, h, pairs):
                    """PV matmuls + reciprocal of the softmax sums.
                    Returns (pa, recip_row) for the deferred norm pass."""
                    pa = psa.tile([P, QB], F32, tag="pa")
                    nmm = 4 * qb + 4
                    mi = 0
                    for (k0, k1, pt, qoffs) in pairs:
                        for i, kt in enumerate((k0, k1)):
                            qoff = qoffs[i]
                            nc.tensor.matmul(
                                pa[0:65, qoff:QB],
                                vaug4[:, h, kt, :],
                                pt[:, i * QB + qoff:(i + 1) * QB],
                                start=(mi == 0),
                                stop=(mi == nmm - 1),
                                skip_group_check=True,
                            )
                            mi += 1
                    sums = recipp.tile([1, QB], F32, tag="sums")
                    nc.vector.tensor_copy(sums[:], pa[64:65, :])
                    recip = recipp.tile([1, QB], F32, tag="recip")
                    nc.vector.reciprocal_approx_fast(recip[:], sums[:])
                    return pa, recip

                # gather-group completion tracking
                normed = {qb: set() for qb in range(NQB)}

                def norm_pass(qb, h, pa, recip):
                    """Broadcast 1/sum, scale, ship to the gather buffer."""
                    gi, row = grp_of[h]
                    bc = bcp.tile([64, QB], F32, tag="bc")
                    nc.gpsimd.partition_broadcast(bc[:], recip[:])
                    atv = atvp.tile([64, QB], BF16, tag="atv")
                    nc.vector.tensor_mul(atv[:], pa[0:64, :], bc[:])
                    nc.sync.dma_start(
                        ag_ins[qb][gi][row * 64:(row + 1) * 64, :], atv[:]
                    )
                    normed[qb].add(h)

                def maybe_gather(qb, h):
                    gi, _ = grp_of[h]
                    s, e = GGRP[gi]
                    if all(x in normed[qb] for x in range(s, e)):
                        nc.gpsimd.collective_compute(
                            "AllGather",
                            mybir.AluOpType.bypass,
                            replica_groups=groups,
                            ins=[ag_ins[qb][gi].opt()],
                            outs=[ag_outs[qb][gi].opt()],
                        )

                def load_agr(qb, gi):
                    # w_out rows are host-permuted to match the gathered
                    # row order [even0-3, odd0-3, even4-5, odd4-5, ...]
                    # issued on the scalar queue: the sync queue carries the
                    # atv DMAs that feed the gathers these loads wait on.
                    ago = ag_outs[qb][gi]
                    ncch = 2 * (GGRP[gi][1] - GGRP[gi][0]) * 64 // P
                    agr = agrp.tile(
                        [P, ncch * QB], BF16,
                        name=f"agr{qb}_{gi}", tag=f"agr{gi}",
                    )
                    nc.scalar.dma_start(
                        agr[:].rearrange("p (c n) -> p c n", n=QB),
                        ago[:].rearrange("(c p) n -> p c n", p=P),
                    )
                    return agr[:].rearrange("p (c n) -> p c n", n=QB)

                def out_proj_oc(qb, agr3s, oc):
                    po = pso.tile([P, QB], F32, tag="po")
                    for cc in range(KC):
                        gi, sub = CCMAP[cc]
                        nc.tensor.matmul(
                            po[:],
                            w23[:, cc, oc * P:(oc + 1) * P],
                            agr3s[gi][:, sub, :],
                            start=(cc == 0),
                            stop=(cc == KC - 1),
                        )
                    osb = outsbp.tile([P, QB], BF16, tag="osb")
                    nc.vector.tensor_scalar_add(
                        osb[:], po[:], bo_sb[:, oc:oc + 1]
                    )
                    nc.sync.dma_start(
                        outT[oc * P:(oc + 1) * P, qb * QB:(qb + 1) * QB],
                        osb[:],
                    )

                # ---- unit pipeline ----
                # qb0 interleaved with qb1; qb3 starts with heads 6,7 so
                # the final out-projection's group-2 gather launches early.
                units = []
                for h in range(HPC):
                    units.append((0, h))
                    units.append((1, h))
                units += [(2, h) for h in range(HPC)]
                units += [(3, h) for h in [6, 7, 0, 1, 2, 3, 4, 5]]

                # step-indexed hooks: agr loads + out-projections for
                # completed query blocks, placed where every gather they
                # consume is long finished; one oc-chunk per step so the
                # single-bank pso pool never head-blocks the tensor queue
                agr_cache = {}

                def hook_agr(qb, gis):
                    def f():
                        lst = agr_cache.setdefault(qb, {})
                        for gi in gis:
                            lst[gi] = load_agr(qb, gi)
                    return f

                def hook_oc(qb, oc):
                    def f():
                        lst = agr_cache[qb]
                        out_proj_oc(qb, [lst[0], lst[1], lst[2]], oc)
                    return f

                hooks = {}
                for k, (pqb, step0) in enumerate([(0, 17), (1, 22), (2, 26)]):
                    hooks.setdefault(step0, []).append(
                        hook_agr(pqb, [0, 1]))
                    hooks.setdefault(step0 + 1, []).append(
                        hook_agr(pqb, [2]))
                    for oc in range(4):
                        hooks.setdefault(step0 + 2 + (oc * 3) // 4, []) \
                            .append(hook_oc(pqb, oc))

                state = {}  # unit -> in-flight data
                for i, u in enumerate(units):
                    # norm of unit i-2 first: its inputs are long ready, so
                    # these ops never head-block their queues
                    if i >= 2:
                        u2 = units[i - 2]
                        pa, recip = state.pop(("pv", u2))
                        norm_pass(u2[0], u2[1], pa, recip)
                    state[("s", u)] = s_pass(*u)
                    if i >= 1:
                        u1 = units[i - 1]
                        state[("pv", u1)] = pv_pass(*u1, state.pop(("s", u1)))
                    if i >= 2:
                        maybe_gather(*units[i - 2])
                    for f in hooks.get(i, ()):
                        f()

                # ---- drain + last out-projection (group-major: only the
                # final group's matmuls sit behind the final gather)
                ulast = units[-1]          # (3, 5)
                usec = units[-2]           # (3, 4)
                state[("pv", ulast)] = pv_pass(*ulast, state.pop(("s", ulast)))
                pa, recip = state.pop(("pv", usec))
                norm_pass(usec[0], usec[1], pa, recip)
                maybe_gather(*usec)

                lagr = {2: load_agr(3, 2), 0: load_agr(3, 0)}
                po_pairs = [
                    pss.tile([P, 2 * QB], F32, tag="ps", name=f"po{i}")
                    for i in range(2)
                ]

                def po_half(oc):
                    return po_pairs[oc // 2][:, (oc % 2) * QB:(oc % 2 + 1) * QB]

                def oproj_group(gi, first, last):
                    chunks = [cc for cc in range(KC) if CCMAP[cc][0] == gi]
                    for oc in range(4):
                        for k, cc in enumerate(chunks):
                            nc.tensor.matmul(
                                po_half(oc),
                                w23[:, cc, oc * P:(oc + 1) * P],
                                lagr[gi][:, CCMAP[cc][1], :],
                                start=(first and k == 0),
                                stop=(last and k == len(chunks) - 1),
                                skip_group_check=True,
                            )

                pa, recip = state.pop(("pv", ulast))
                norm_pass(ulast[0], ulast[1], pa, recip)
                maybe_gather(*ulast)          # final gather (g1, heads 4-5)
                oproj_group(2, True, False)   # overlaps the final gather
                oproj_group(0, False, False)
                lagr[1] = load_agr(3, 1)
                oproj_group(1, False, True)
                for oc in range(4):
                    osb = outsbp.tile([P, QB], BF16, tag="osb")
                    nc.vector.tensor_scalar_add(
                        osb[:], po_half(oc), bo_sb[:, oc:oc + 1]
                    )
                    nc.sync.dma_start(
                        outT[oc * P:(oc + 1) * P, 3 * QB:4 * QB],
                        osb[:],
                    )

    nc.compile()
    return nc


def kernel(x, w_qkv, b_qkv, w_out, b_out):
    x = np.asarray(x, dtype=np.float32)
    w_qkv = np.asarray(w_qkv, dtype=np.float32)
    b_qkv = np.asarray(b_qkv, dtype=np.float32)
    w_out = np.asarray(w_out, dtype=np.float32)
    b_out = np.asarray(b_out, dtype=np.float32)

    if "nc" not in _CACHE:
        _CACHE["nc"] = build()
    nc = _CACHE["nc"]

    # V bias passes through softmax unchanged; fold it into the out bias
    bv_all = b_qkv[2 * C:3 * C]

    in_maps = []
    for c in range(NCORES):
        b = c // 2
        h0 = (c % 2) * HPC
        cols = slice(h0 * D, h0 * D + CPC)
        wq = np.concatenate(
            [w_qkv[:, cols], w_qkv[:, C:][:, cols], w_qkv[:, 2 * C:][:, cols]],
            axis=1,
        )
        bq = np.concatenate(
            [b_qkv[cols], b_qkv[C:][cols], b_qkv[2 * C:][cols]]
        )
        # per-partition layout: bq_sb[p, n] = bq[n*128 + p]
        bq_pp = np.ascontiguousarray(bq.reshape(12, P).T)
        half = slice((c % 2) * CPC, (c % 2) * CPC + CPC)
        wo = w_out[:, half]
        # rows permuted to the gathered channel order:
        # [even h0-3, odd h0-3, even h4-5, odd h4-5, even h6-7, odd h6-7]
        wo_perm = np.concatenate(
            [wo[0:256], wo[512:768],
             wo[256:384], wo[768:896],
             wo[384:512], wo[896:1024]], axis=0
        )
        bout_eff = b_out[half] + bv_all @ w_out[:, half]
        bo_pp = np.ascontiguousarray(bout_eff.reshape(4, P).T)
        in_maps.append({
            "xT": np.ascontiguousarray(x[b].T.astype(ml_dtypes.bfloat16)),
            "wqkv": np.ascontiguousarray(wq.astype(ml_dtypes.bfloat16)),
            "bqkv": bq_pp,
            "wout": np.ascontiguousarray(wo_perm.astype(ml_dtypes.bfloat16)),
            "bout": bo_pp,
        })

    kwargs = {}
    tdir = os.environ.get("KERNEL_TRACE_DIR")
    if tdir:
        kwargs = dict(trace=True, tmpdir=tdir)
    res = run_bass_kernel_spmd(
        nc, in_maps, core_ids=list(range(NCORES)), **kwargs
    )
    _CACHE["last_results"] = res

    out = np.empty((B, T, C), dtype=np.float32)
    for c in range(NCORES):
        b = c // 2
        half = slice((c % 2) * CPC, (c % 2) * CPC + CPC)
        out[b][:, half] = res.results[c]["outT"].T.astype(np.float32)
    return out
